# revision 3
# baseline (speedup 1.0000x reference)
"""CubePad Trainium2 kernel, v2 — SBUF-assembled, big-descriptor DMA.

Input  x: [12, 64, 256, 256] f32  (2 cubes x 6 faces, face order F,R,B,L,T,D)
Output y: [12, 64, 258, 258] f32  (1-px border gathered from neighboring faces)

Sharding: channel-parallel across 8 cores (8 channels each); every core holds
all 12 faces. Per core, 8 chunks of (cube, 2 channels) are pipelined:

  SP   : HBM loads (6144B descs) -> xt, s0-row SBUF shift, HBM stores
         (6192B descs: 258 rows = 43 partitions x 6 rows exactly)
  DVE/Pool (E1): interior row-shift copies, col strip slot-shifts,
         column compactions, in-partition reversals (engines allow negative
         strides; DMA SBUF APs do NOT — that crashes real HW)
  ACT (D1): small SBUF->SBUF strip DMAs + staging landings (all fwd strides)
  DVE (E2): reversals of staged rows (partition 0/64 only — engine ops must
         start at partition 0/32/64/96)
  ACT (D2): final placement of reversed rows + corner duplication

SBUF layout per channel block b (partition base 64*b, 43 partitions each):
  xt[64b+p, f, s, w] = x[face f, row 6p+s, col w]   (s<6, rows 252-255 pad)
  yt[64b+p, f, s, e] = y[face f, row 6p+s, col e]   (258 rows = 43*6 exact)
"""

import numpy as np

N_CORES = 8
NF, C_FULL, H, W = 12, 64, 256, 256
C = C_FULL // N_CORES  # 8 channels per core
HP, WP = H + 2, W + 2
P = 43   # partitions per channel block
PB = 64  # partition base stride between channel blocks
NCHUNK = 8  # (2 cubes) x (4 channel pairs)
NPART = PB + P  # 107

# face indices within a cube
F, R, B, L, T, D = 0, 1, 2, 3, 4, 5

REV = slice(255, None, -1)  # engine-only; never in DMA APs


def _chunk_ops():
    """Op tables for one chunk (identical for every chunk; only DRAM offsets
    differ). Index tuples apply identically to numpy arrays and bass handles.
      xt: [107,6,6,256]  yt: [107,6,6,258]  sc: [107,4,6]  st: [107,10,256]
    Each op = (dst_name, dst_idx, src_name, src_idx).
    """
    e1v, e1p, d1, e2, d2, corner, s0 = [], [], [], [], [], [], []

    for b in (0, 1):
        o = b * PB
        p0, p42 = o, o + 42
        P0 = slice(p0, p0 + 1)
        P42 = slice(p42, p42 + 1)

        # ---- E1 interior: out row R=6p+s (cols 1..256) <- x row R-1 ----
        # s=1..5, but (p42, s5) (= row 257) excluded: s1-4 full, s5 p<42.
        for f in range(6):
            lst = e1v if f < 3 else e1p
            lst.append(("yt", (slice(o, o + P), f, slice(1, 5), slice(1, 257)),
                        "xt", (slice(o, o + P), f, slice(0, 4), slice(0, 256))))
            lst.append(("yt", (slice(o, o + 42), f, 5, slice(1, 257)),
                        "xt", (slice(o, o + 42), f, 4, slice(0, 256))))

        # ---- E1 (E): col->col strips, partition-aligned slot shift ----
        # l of faces 0..3 <- col 255 of [L,F,R,B] ; r <- col 0 of [R,B,L,F]
        for i, sf in enumerate([L, F, R, B]):
            e1v.append(("yt", (slice(o, o + P), i, slice(1, 5), 0),
                        "xt", (slice(o, o + P), sf, slice(0, 4), 255)))
            e1v.append(("yt", (slice(o, o + 42), i, slice(5, 6), slice(0, 1)),
                        "xt", (slice(o, o + 42), sf, slice(4, 5), slice(255, 256))))
        for i, sf in enumerate([R, B, L, F]):
            e1p.append(("yt", (slice(o, o + P), i, slice(1, 5), 257),
                        "xt", (slice(o, o + P), sf, slice(0, 4), 0)))
            e1p.append(("yt", (slice(o, o + 42), i, slice(5, 6), slice(257, 258)),
                        "xt", (slice(o, o + 42), sf, slice(4, 5), slice(0, 1))))

        # ---- E1 (C/D): column compactions into scratch ----
        # sc slots: 0 = T col0 (t of f3), 1 = D col255 (d of f1),
        #           2 = T col255 (t of f1, reversed later), 3 = D col0 (d of f3)
        e1v.append(("sc", (slice(o, o + P), 0, slice(0, 6)),
                    "xt", (slice(o, o + P), T, slice(0, 6), 0)))
        e1v.append(("sc", (slice(o, o + P), 2, slice(0, 6)),
                    "xt", (slice(o, o + P), T, slice(0, 6), 255)))
        e1p.append(("sc", (slice(o, o + P), 1, slice(0, 6)),
                    "xt", (slice(o, o + P), D, slice(0, 6), 255)))
        e1p.append(("sc", (slice(o, o + P), 3, slice(0, 6)),
                    "xt", (slice(o, o + P), D, slice(0, 6), 0)))

        # ---- E1 (B)-top: reversed rows, dst partition 0/64 (aligned) ----
        # t of 2 <- T row0 rev ; t of 4 <- B row0 rev
        e1v.append(("yt", (P0, 2, 0, slice(1, 257)), "xt", (P0, T, 0, REV)))
        e1v.append(("yt", (P0, 4, 0, slice(1, 257)), "xt", (P0, B, 0, REV)))
        # ---- E1 (G)-r4: reverse R row0 into staging c5 ----
        e1p.append(("st", (P0, 5, slice(0, 256)), "xt", (P0, R, 0, REV)))

        # ================= D1 (ACT): forward-stride DMAs =================
        # (A) row->row strips, cross-partition, 1KB descs
        d1.append(("yt", (P0, 0, 0, slice(1, 257)), "xt", (P42, T, 3, slice(0, 256))))
        d1.append(("yt", (P0, 5, 0, slice(1, 257)), "xt", (P42, F, 3, slice(0, 256))))
        d1.append(("yt", (P42, 0, 5, slice(1, 257)), "xt", (P0, D, 0, slice(0, 256))))
        d1.append(("yt", (P42, 4, 5, slice(1, 257)), "xt", (P0, F, 0, slice(0, 256))))

        # (B)-down landings: D row255, B row255 -> staging c6, c8 (1KB)
        d1.append(("st", (P0, 6, slice(0, 256)), "xt", (P42, D, 3, slice(0, 256))))
        d1.append(("st", (P0, 8, slice(0, 256)), "xt", (P42, B, 3, slice(0, 256))))

        # (C-dst) compacted col -> row, 24B descs
        d1.append(("yt", (P0, 3, 0, slice(1, 253)), "sc", (slice(o, o + 42), 0, slice(0, 6))))
        d1.append(("yt", (P0, 3, 0, slice(253, 257)), "sc", (P42, 0, slice(0, 4))))
        d1.append(("yt", (P42, 1, 5, slice(1, 253)), "sc", (slice(o, o + 42), 1, slice(0, 6))))
        d1.append(("yt", (P42, 1, 5, slice(253, 257)), "sc", (P42, 1, slice(0, 4))))

        # (D) landings: compacted cols -> staging rows c0, c1 (24B descs)
        d1.append(("st", (P0, 0, slice(0, 252)), "sc", (slice(o, o + 42), 2, slice(0, 6))))
        d1.append(("st", (P0, 0, slice(252, 256)), "sc", (P42, 2, slice(0, 4))))
        d1.append(("st", (P0, 1, slice(0, 252)), "sc", (slice(o, o + 42), 3, slice(0, 6))))
        d1.append(("st", (P0, 1, slice(252, 256)), "sc", (P42, 3, slice(0, 4))))

        # (E-s0) col strips, s=0 slots (cross-partition by one)
        for i, sf in enumerate([L, F, R, B]):
            d1.append(("yt", (slice(o + 1, o + 43), i, slice(0, 1), slice(0, 1)),
                       "xt", (slice(o, o + 42), sf, slice(5, 6), slice(255, 256))))
        for i, sf in enumerate([R, B, L, F]):
            d1.append(("yt", (slice(o + 1, o + 43), i, slice(0, 1), slice(257, 258)),
                       "xt", (slice(o, o + 42), sf, slice(5, 6), slice(0, 1))))

        # (F)/(G) scatters: row -> col (4B descs, forward only)
        def scatter(lst, face, col, sn, sidx_of):
            cs = slice(col, col + 1)
            lst.append(("yt", (slice(o, o + 1), face, slice(1, 6), cs),
                        sn, sidx_of(slice(0, 5))))
            lst.append(("yt", (slice(o + 1, o + 42), face, slice(0, 6), cs),
                        sn, sidx_of(slice(5, 251))))
            lst.append(("yt", (P42, face, slice(0, 5), cs),
                        sn, sidx_of(slice(251, 256))))

        # (F) l of face 4 <- L row0 ; r of face 5 <- R row255 (fwd from xt)
        scatter(d1, 4, 0, "xt", lambda s: (P0, L, 0, s))
        scatter(d1, 5, 257, "xt", lambda s: (P42, R, 3, s))
        # (G) r of face 4 <- reversed R row0 (staged in c5 by E1)
        scatter(d1, 4, 257, "st", lambda s: (P0, 5, s))
        # (G) l of face 5 landing: L row255 -> staging c3
        d1.append(("st", (P0, 3, slice(0, 256)), "xt", (P42, L, 3, slice(0, 256))))

        # ================= E2 (DVE): staged reversals ====================
        # t of f1 <- reverse(c0) directly into yt row 0
        e2.append(("yt", (P0, 1, 0, slice(1, 257)), "st", (P0, 0, REV)))
        e2.append(("st", (P0, 2, slice(0, 256)), "st", (P0, 1, REV)))  # d of f3
        e2.append(("st", (P0, 4, slice(0, 256)), "st", (P0, 3, REV)))  # l of f5
        e2.append(("st", (P0, 7, slice(0, 256)), "st", (P0, 6, REV)))  # d of f2
        e2.append(("st", (P0, 9, slice(0, 256)), "st", (P0, 8, REV)))  # d of f5

        # ================= D2 (ACT): final placements ====================
        d2.append(("yt", (P42, 3, 5, slice(1, 257)), "st", (P0, 2, slice(0, 256))))
        d2.append(("yt", (P42, 2, 5, slice(1, 257)), "st", (P0, 7, slice(0, 256))))
        d2.append(("yt", (P42, 5, 5, slice(1, 257)), "st", (P0, 9, slice(0, 256))))
        scatter(d2, 5, 0, "st", lambda s: (P0, 4, s))  # l of face 5

        # ---- corners (after D2): dup row 0/257 endpoints ----
        for pp, ss in ((p0, 0), (p42, 5)):
            corner.append(("yt", (slice(pp, pp + 1), slice(0, 6), ss, slice(0, 1)),
                           "yt", (slice(pp, pp + 1), slice(0, 6), ss, slice(1, 2))))
            corner.append(("yt", (slice(pp, pp + 1), slice(0, 6), ss, slice(257, 258)),
                           "yt", (slice(pp, pp + 1), slice(0, 6), ss, slice(256, 257))))

        # ---- s0 interior rows (SP, SBUF->SBUF 1KB descs) ----
        s0.append(("yt", (slice(o + 1, o + 43), slice(0, 6), 0, slice(1, 257)),
                   "xt", (slice(o, o + 42), slice(0, 6), 5, slice(0, 256))))

    return dict(e1v=e1v, e1p=e1p, d1=d1, e2=e2, d2=d2, corner=corner, s0=s0)


_OPS = _chunk_ops()

N_LOAD = 4
N_S0 = len(_OPS["s0"])      # 2
N_STORE = 2
N_D1 = len(_OPS["d1"])      # 64
N_D2 = len(_OPS["d2"])      # 12
N_CN = len(_OPS["corner"])  # 8
N_E1V = len(_OPS["e1v"])    # 36
N_E1P = len(_OPS["e1p"])    # 34
N_E2 = len(_OPS["e2"])      # 10


def _build_bass():
    import dataclasses

    import concourse.bass as bass
    import concourse.mybir as mybir

    def dap(t, offset, dims):
        """Custom DRAM-side AP: element-space [step, count] dims."""
        ap = t[tuple(slice(None) for _ in t.shape)]
        return dataclasses.replace(ap, offset=offset, ap=[[s, c] for s, c in dims])

    nc = bass.Bass()
    dt = mybir.dt.float32
    x = nc.dram_tensor("x", [NF, C, H, W], dt, kind="ExternalInput")
    y = nc.dram_tensor("y", [NF, C, HP, WP], dt, kind="ExternalOutput")

    xt = [nc.alloc_sbuf_tensor(f"xt{v}", [NPART, 6, 6, 256], dt) for v in range(2)]
    yt = [nc.alloc_sbuf_tensor(f"yt{v}", [NPART, 6, 6, 258], dt) for v in range(2)]
    sc = [nc.alloc_sbuf_tensor(f"sc{v}", [NPART, 4, 6], dt) for v in range(2)]
    st = [nc.alloc_sbuf_tensor(f"st{v}", [NPART, 10, 256], dt) for v in range(2)]

    sem_l = nc.alloc_semaphore("sem_l")
    sem_l2 = nc.alloc_semaphore("sem_l2")
    sem_s = nc.alloc_semaphore("sem_s")
    sem_d = nc.alloc_semaphore("sem_d")
    sem_d2 = nc.alloc_semaphore("sem_d2")
    sem_c = nc.alloc_semaphore("sem_c")
    sem_e1v = nc.alloc_semaphore("sem_e1v")
    sem_e1p = nc.alloc_semaphore("sem_e1p")
    sem_e2 = nc.alloc_semaphore("sem_e2")
    sem_ms = nc.alloc_semaphore("sem_ms")

    chunks = [(k // 4, 2 * (k % 4)) for k in range(NCHUNK)]

    def tens(v):
        return {"xt": xt[v], "yt": yt[v], "sc": sc[v], "st": st[v]}

    def emit(engine, v, op):
        dn, di, sn, si = op
        tt = tens(v)
        return engine.tensor_copy(tt[dn][di], tt[sn][si])

    def emit_dma(engine, v, op):
        dn, di, sn, si = op
        tt = tens(v)
        return engine.dma_start(tt[dn][di], tt[sn][si])

    def store_chunk(sync, k):
        v = k % 2
        cube, ch0 = chunks[k]
        fb = cube * 6
        sync.wait_ge(sem_c, N_CN * 16 * (k + 1))
        sync.wait_ge(sem_l2, N_S0 * 16 * (k + 1))
        for b in (0, 1):
            sync.dma_start(
                dap(y, (fb * C + ch0 + b) * HP * WP,
                    [(6 * WP, P), (C * HP * WP, 6), (1, 6 * WP)]),
                yt[v][b * PB:b * PB + P, :, :, :],
            ).then_inc(sem_s, 16)

    with nc.Block() as blk:

        @blk.sync
        def _(sync):
            with nc.allow_non_contiguous_dma(reason="cubepad sbuf strips"):
                sync.wait_ge(sem_ms, 2)  # xt memset done
                for k in range(NCHUNK):
                    v = k % 2
                    cube, ch0 = chunks[k]
                    fb = cube * 6
                    if k >= 2:
                        sync.wait_ge(sem_c, N_CN * 16 * (k - 1))
                    for b in (0, 1):
                        ch = ch0 + b
                        o = b * PB
                        sync.dma_start(
                            xt[v][o:o + 42, :, :, :],
                            dap(x, (fb * C + ch) * H * W,
                                [(6 * W, 42), (C * H * W, 6), (1, 6 * W)]),
                        ).then_inc(sem_l, 16)
                        sync.dma_start(
                            xt[v][o + 42:o + 43, :, 0:4, :],
                            dap(x, ((fb * C + ch) * H + 252) * W,
                                [(C * H * W, 6), (1, 4 * W)]),
                        ).then_inc(sem_l, 16)
                    sync.wait_ge(sem_l, N_LOAD * 16 * (k + 1))
                    if k >= 2:
                        sync.wait_ge(sem_s, N_STORE * 16 * (k - 1))
                    for op in _OPS["s0"]:
                        emit_dma(sync, v, op).then_inc(sem_l2, 16)
                    if k >= 1:
                        store_chunk(sync, k - 1)
                store_chunk(sync, NCHUNK - 1)
                sync.wait_ge(sem_s, N_STORE * 16 * NCHUNK)

        @blk.vector
        def _(vector):
            for v in range(2):
                vector.memset(xt[v][:, :, :, :], 0).then_inc(sem_ms, 1)
            for k in range(NCHUNK):
                v = k % 2
                vector.wait_ge(sem_l, N_LOAD * 16 * (k + 1))
                if k >= 2:
                    vector.wait_ge(sem_s, N_STORE * 16 * (k - 1))
                    vector.wait_ge(sem_c, N_CN * 16 * (k - 1))
                for op in _OPS["e1v"]:
                    emit(vector, v, op).then_inc(sem_e1v, 1)
                vector.wait_ge(sem_d, N_D1 * 16 * (k + 1))
                for op in _OPS["e2"]:
                    emit(vector, v, op).then_inc(sem_e2, 1)

        @blk.gpsimd
        def _(gp):
            for k in range(NCHUNK):
                v = k % 2
                gp.wait_ge(sem_l, N_LOAD * 16 * (k + 1))
                if k >= 2:
                    gp.wait_ge(sem_s, N_STORE * 16 * (k - 1))
                    gp.wait_ge(sem_c, N_CN * 16 * (k - 1))
                for op in _OPS["e1p"]:
                    emit(gp, v, op).then_inc(sem_e1p, 1)

        @blk.scalar
        def _(scalar):
            with nc.allow_non_contiguous_dma(reason="cubepad sbuf strips"):
                for k in range(NCHUNK):
                    v = k % 2
                    scalar.wait_ge(sem_e1v, N_E1V * (k + 1))
                    scalar.wait_ge(sem_e1p, N_E1P * (k + 1))
                    for op in _OPS["d1"]:
                        emit_dma(scalar, v, op).then_inc(sem_d, 16)
                    scalar.wait_ge(sem_e2, N_E2 * (k + 1))
                    for op in _OPS["d2"]:
                        emit_dma(scalar, v, op).then_inc(sem_d2, 16)
                    scalar.wait_ge(sem_d2, N_D2 * 16 * (k + 1))
                    for op in _OPS["corner"]:
                        emit_dma(scalar, v, op).then_inc(sem_c, 16)

    nc.finalize()
    return nc


# ---------------------------------------------------------------- emulator
def emulate(x_core):
    """Numpy emulation of the exact op schedule (dev-time validation)."""
    assert x_core.shape == (NF, C, H, W)
    y_full = np.full((NF, C, HP, WP), np.nan, dtype=np.float32)
    XT = np.zeros((2, NPART, 6, 6, 256), dtype=np.float32)  # memset
    YT = np.full((2, NPART, 6, 6, 258), np.nan, dtype=np.float32)
    SC = np.full((2, NPART, 4, 6), np.nan, dtype=np.float32)
    ST = np.full((2, NPART, 10, 256), np.nan, dtype=np.float32)

    for k in range(NCHUNK):
        v = k % 2
        cube, j = divmod(k, 4)
        ch0 = 2 * j
        fb = cube * 6
        tt = {"xt": XT[v], "yt": YT[v], "sc": SC[v], "st": ST[v]}
        for b in (0, 1):
            ch = ch0 + b
            o = b * PB
            src = x_core[fb:fb + 6, ch, 0:252, :].reshape(6, 42, 6, 256)
            XT[v][o:o + 42] = src.transpose(1, 0, 2, 3)
            src = x_core[fb:fb + 6, ch, 252:256, :].reshape(6, 1, 4, 256)
            XT[v][o + 42:o + 43, :, 0:4, :] = src.transpose(1, 0, 2, 3)
        for phase in ("e1v", "e1p", "s0", "d1", "e2", "d2", "corner"):
            for dn, di, sn, si in _OPS[phase]:
                dst = tt[dn][di]
                tt[dn][di] = tt[sn][si].reshape(dst.shape)
        for b in (0, 1):
            o = b * PB
            y_full[fb:fb + 6, ch0 + b] = YT[v][o:o + P].transpose(1, 0, 2, 3).reshape(6, HP, WP)
    return y_full


# ---------------------------------------------------------------- entry
_NC_CACHE = None
_TRACE = False
_LAST_EXEC_NS = None


def kernel(x: np.ndarray) -> np.ndarray:
    global _NC_CACHE, _LAST_EXEC_NS
    from concourse.bass_utils import run_bass_kernel_spmd

    assert x.shape == (NF, C_FULL, H, W) and x.dtype == np.float32
    if _NC_CACHE is None:
        _NC_CACHE = _build_bass()
    nc = _NC_CACHE

    in_maps = [
        {"x": np.ascontiguousarray(x[:, i * C:(i + 1) * C])} for i in range(N_CORES)
    ]
    res = run_bass_kernel_spmd(
        nc, in_maps, core_ids=list(range(N_CORES)), trace=_TRACE
    )
    _LAST_EXEC_NS = res.exec_time_ns
    out = np.empty((NF, C_FULL, HP, WP), dtype=np.float32)
    for i in range(N_CORES):
        out[:, i * C:(i + 1) * C] = res.results[i]["y"]
    return out
